# revision 1
# baseline (speedup 1.0000x reference)
"""Trainium2 Bass kernel for nn_CumulHazardFunctionNetwork.

Per token (b, s) with hidden state h [256] and time delta td:
    o1  = tanh(h @ w2h.T + td*u1 + c1)          u1 = w2t @ w1[:,0],  c1 = w2t@b1 + b2
    o2  = tanh(o1 @ wA.T + bA)
    o3  = tanh(o2 @ wB.T + bB)
    y   = softplus(o3 @ w3.T + b3)              (integral_lambda)
    yd  = sigmoid(z4) * (d3 @ w3.T) / (B*S)     (derivative_integral_lambda, JVP wrt td)
with the JVP chain d1 = u1s*(1-o1^2), d_{k+1} = tanh'(z_{k+1}) * (d_k @ W.T).

Mapping: pure data parallel over 8 NeuronCores (16384 tokens each),
feature-major layout ([feature, token]); weights replicated; fp32r matmuls.

z2, z3 >> 0 always (all-positive weights), so layers 2/3 use
    e = exp(-2*z),  tanh(z) = 1 - 2e,  tanh'(z) = 4e     (rel err < 3e-6)
and the "1 - 2e" is folded into the next matmul:
    z3 = (rowsum(wB)+bB) (x) 1  -  2*e2 @ wB.T
    z4 = (rowsum(w3)+b3) (x) 1  -  2*e3 @ w3.T
so o2/o3 are never materialized. This avoids the catastrophic cancellation of
1 - tanh^2 and lands the derivative output on the fp32 reference's own noise
floor. softplus/sigmoid run in a second phase (one ACT table-set switch) on
the repacked [10, T] tail: y = ln(1+e^z4), sigma = exp(z4 - y).
"""

import numpy as np

H, K, B, S = 256, 10, 32, 4096
BS = B * S
NCORES = 8
TCORE = BS // NCORES          # 16384 tokens per core
NT = 512                      # tokens per block (PSUM bank = 512 fp32)
NT2 = 2 * NT
NBLK = TCORE // NT            # 32
FLAT = NBLK * K * NT          # 163840 = 128 * 1280
TDC = 4                       # blocks per tdo chunk
HB = 2                        # blocks per h DMA
TB = 2                        # blocks per tail DMA

_NC_CACHE = {}


def _build_nc(repeat=1):
    import concourse.mybir as mybir
    import concourse.tile as tile
    from concourse import bacc

    dt = mybir.dt
    f32 = dt.float32
    f32r = dt.float32r
    A = mybir.ActivationFunctionType
    Op = mybir.AluOpType

    nc = bacc.Bacc("TRN2", target_bir_lowering=False, debug=False)

    h_t = nc.dram_tensor("h_t", [2, 128, TCORE], f32r, kind="ExternalInput")
    # row 0: ones, row 1: td  (rank-1 rhs for [c1; u1]; ones at partition 0)
    tdo_in = nc.dram_tensor("tdo_in", [2, TCORE], f32r, kind="ExternalInput")
    w2hT = nc.dram_tensor("w2hT", [2, 128, H], f32r, kind="ExternalInput")
    wAT = nc.dram_tensor("wAT", [2, 128, H], f32r, kind="ExternalInput")
    wBn2T = nc.dram_tensor("wBn2T", [2, 128, H], f32r, kind="ExternalInput")   # (-2*wB).T
    w3n2T = nc.dram_tensor("w3n2T", [2, 128, K], f32r, kind="ExternalInput")   # (-2*w3).T
    w3T = nc.dram_tensor("w3T", [2, 128, K], f32r, kind="ExternalInput")
    u1c1 = nc.dram_tensor("u1c1", [2, H], f32r, kind="ExternalInput")          # [c1; u1]
    cB3_d = nc.dram_tensor("cB3_d", [1, H], f32r, kind="ExternalInput")        # rowsum(wB)+bB
    c34_d = nc.dram_tensor("c34_d", [1, K], f32r, kind="ExternalInput")        # rowsum(w3)+b3
    cvec = nc.dram_tensor("cvec", [128, 4], f32, kind="ExternalInput")         # -2*bA, -2*(rowsum(wB)+bB) per m
    u1s_d = nc.dram_tensor("u1s_d", [128, 2], f32, kind="ExternalInput")
    y_out = nc.dram_tensor("y_out", [FLAT], f32, kind="ExternalOutput")
    yd_out = nc.dram_tensor("yd_out", [FLAT], f32, kind="ExternalOutput")

    with tile.TileContext(nc) as tc:
        with tc.tile_pool(name="consts", bufs=1) as consts, \
             tc.tile_pool(name="hp", bufs=3) as hp, \
             tc.tile_pool(name="tdp", bufs=2) as tdp, \
             tc.tile_pool(name="ap", bufs=3) as ap, \
             tc.tile_pool(name="tailp", bufs=2) as tailp, \
             tc.tile_pool(name="ph2", bufs=1) as ph2, \
             tc.tile_pool(name="ps", bufs=4, space="PSUM") as psp, \
             tc.tile_pool(name="scr", bufs=1, space="DRAM") as scr:

            def body():
                def wtile(name, src, fdim):
                    t = consts.tile([128, 2, fdim], f32r, tag=name)
                    nc.sync.dma_start(out=t[:], in_=src[:].rearrange("k p f -> p k f"))
                    return t

                w2hT_sb = wtile("w2hT", w2hT, H)
                wAT_sb = wtile("wAT", wAT, H)
                wBn2T_sb = wtile("wBn2T", wBn2T, H)
                w3n2T_sb = wtile("w3n2T", w3n2T, K)
                w3T_sb = wtile("w3T", w3T, K)
                u1c1_sb = consts.tile([2, H], f32r, tag="u1c1")
                nc.sync.dma_start(out=u1c1_sb[:], in_=u1c1[:])
                c34_sb = consts.tile([1, K], f32r, tag="c34")
                nc.sync.dma_start(out=c34_sb[:], in_=c34_d[:])
                cvec_sb = consts.tile([128, 4], f32, tag="cvec")
                nc.sync.dma_start(out=cvec_sb[:], in_=cvec[:])
                u1s_sb = consts.tile([128, 2], f32, tag="u1s")
                nc.sync.dma_start(out=u1s_sb[:], in_=u1s_d[:])

                z4_s = scr.tile([FLAT], f32, tag="z4s")
                p_s = scr.tile([FLAT], f32, tag="ps_scr")

                h_sb = None
                tdo_sb = None
                tsb4 = None
                for j in range(NBLK):
                    if j % HB == 0:
                        h_sb = hp.tile([128, 2, HB * NT], f32r, tag="h")
                        nc.sync.dma_start(
                            out=h_sb[:],
                            in_=h_t[:, :, j * NT:(j + HB) * NT].rearrange("k p n -> p k n"))
                    if j % TDC == 0:
                        tdo_sb = tdp.tile([2, TDC * NT], f32r, tag="tdo")
                        nc.sync.dma_start(
                            out=tdo_sb[:], in_=tdo_in[:, j * NT:(j + TDC) * NT])
                    hs = [h_sb[:, kk, (j % HB) * NT:(j % HB + 1) * NT] for kk in range(2)]
                    tds = tdo_sb[:, (j % TDC) * NT:(j % TDC + 1) * NT]

                    # ---- layer 1: z1 = h @ w2h.T + td*u1 + c1 (K=2 rank-1)
                    z1 = psp.tile([128, 2, NT], f32, tag="ps")
                    for m in range(2):
                        ms = slice(m * 128, (m + 1) * 128)
                        nc.tensor.matmul(z1[:, m, :], w2hT_sb[:, 0, ms], hs[0],
                                         start=True, stop=False)
                        nc.tensor.matmul(z1[:, m, :], w2hT_sb[:, 1, ms], hs[1],
                                         start=False, stop=False)
                        nc.tensor.matmul(z1[:, m, :], u1c1_sb[:, ms], tds,
                                         start=False, stop=True)
                    o1 = ap.tile([128, 2, NT], f32r, tag="o1")
                    nc.scalar.activation(o1[:].rearrange("p k n -> p (k n)"),
                                         z1[:].rearrange("p k n -> p (k n)"), A.Tanh)
                    # d1 = (u1s + u1s*o1) * (1 - o1), carried negated
                    d1n = ap.tile([128, 2, NT], f32r, tag="d1n")
                    for m in range(2):
                        o1f = o1[:, m, :].bitcast(f32)
                        t1 = ap.tile([128, NT], f32, tag=f"t1_{m}")
                        nc.vector.tensor_scalar(t1[:], o1f, u1s_sb[:, m:m + 1],
                                                u1s_sb[:, m:m + 1], Op.mult, Op.add)
                        nc.vector.scalar_tensor_tensor(d1n[:, m, :], o1f, 1.0, t1[:],
                                                       Op.subtract, Op.mult)  # -d1

                    # ---- layer 2: z2 = o1 @ wA.T ; e2 = exp(-2*(z2+bA))
                    z2 = psp.tile([128, 2, NT], f32, tag="ps")
                    g2n = psp.tile([128, 2, NT], f32, tag="ps")
                    for m in range(2):
                        ms = slice(m * 128, (m + 1) * 128)
                        nc.tensor.matmul(z2[:, m, :], wAT_sb[:, 0, ms], o1[:, 0, :],
                                         start=True, stop=False)
                        nc.tensor.matmul(z2[:, m, :], wAT_sb[:, 1, ms], o1[:, 1, :],
                                         start=False, stop=True)
                        nc.tensor.matmul(g2n[:, m, :], wAT_sb[:, 0, ms], d1n[:, 0, :],
                                         start=True, stop=False)
                        nc.tensor.matmul(g2n[:, m, :], wAT_sb[:, 1, ms], d1n[:, 1, :],
                                         start=False, stop=True)
                    e2 = ap.tile([128, 2, NT], f32r, tag="e2")
                    for m in range(2):
                        nc.scalar.activation(e2[:, m, :], z2[:, m, :], A.Exp,
                                             scale=-2.0, bias=cvec_sb[:, m:m + 1])
                    d2n = ap.tile([128, 2, NT], f32r, tag="d2n")
                    nc.vector.scalar_tensor_tensor(
                        d2n[:].rearrange("p k n -> p (k n)"),
                        e2[:].rearrange("p k n -> p (k n)").bitcast(f32), 4.0,
                        g2n[:].rearrange("p k n -> p (k n)"), Op.mult, Op.mult)  # -d2

                    # ---- layer 3: z3 = (rowsum(wB)+bB)(x)1 - 2*e2 @ wB.T
                    z3 = psp.tile([128, 2, NT], f32, tag="ps")
                    g3n = psp.tile([128, 2, NT], f32, tag="ps")
                    for m in range(2):
                        ms = slice(m * 128, (m + 1) * 128)
                        nc.tensor.matmul(z3[:, m, :], wBn2T_sb[:, 0, ms], e2[:, 0, :],
                                         start=True, stop=False)
                        nc.tensor.matmul(z3[:, m, :], wBn2T_sb[:, 1, ms], e2[:, 1, :],
                                         start=False, stop=True)
                        nc.tensor.matmul(g3n[:, m, :], wBn2T_sb[:, 0, ms], d2n[:, 0, :],
                                         start=True, stop=False)
                        nc.tensor.matmul(g3n[:, m, :], wBn2T_sb[:, 1, ms], d2n[:, 1, :],
                                         start=False, stop=True)
                    e3 = ap.tile([128, 2, NT], f32r, tag="e3")
                    for m in range(2):
                        nc.scalar.activation(e3[:, m, :], z3[:, m, :], A.Exp,
                                             scale=-2.0, bias=cvec_sb[:, 2 + m:3 + m])
                    # g3n = (-2*wB).T-mm of d2n = (-2)*(-g3) = +2*g3 ; d3 = 4*e3*g3
                    # so d3n = -d3 = -2 * e3 * g3n
                    d3n = ap.tile([128, 2, NT], f32r, tag="d3n")
                    nc.vector.scalar_tensor_tensor(
                        d3n[:].rearrange("p k n -> p (k n)"),
                        e3[:].rearrange("p k n -> p (k n)").bitcast(f32), -2.0,
                        g3n[:].rearrange("p k n -> p (k n)"), Op.mult, Op.mult)

                    # ---- tail: z4 = (rowsum(w3)+b3)(x)1 - 2*e3 @ w3.T ; pn = d3n @ w3.T
                    tl = psp.tile([K, 2, NT], f32, tag="ps")
                    nc.tensor.matmul(tl[:, 0, :], w3n2T_sb[:, 0, :], e3[:, 0, :],
                                     start=True, stop=False)
                    nc.tensor.matmul(tl[:, 0, :], w3n2T_sb[:, 1, :], e3[:, 1, :],
                                     start=False, stop=False)
                    nc.tensor.matmul(tl[:, 0, :], c34_sb[:], tdo_sb[0:1, (j % TDC) * NT:(j % TDC + 1) * NT],
                                     start=False, stop=True)
                    nc.tensor.matmul(tl[:, 1, :], w3T_sb[:, 0, :], d3n[:, 0, :],
                                     start=True, stop=False)
                    nc.tensor.matmul(tl[:, 1, :], w3T_sb[:, 1, :], d3n[:, 1, :],
                                     start=False, stop=True)
                    if j % TB == 0:
                        tsb4 = tailp.tile([K, TB, 2, NT], f32, tag="tail")
                    nc.vector.tensor_copy(
                        tsb4[:, j % TB, :, :].rearrange("p k n -> p (k n)"),
                        tl[:].rearrange("p k n -> p (k n)"))
                    if j % TB == TB - 1:
                        j0 = j - (TB - 1)
                        rng = slice(j0 * K * NT, (j0 + TB) * K * NT)
                        nc.sync.dma_start(
                            out=z4_s[rng].rearrange("(b r c) -> r b c", r=K, c=NT),
                            in_=tsb4[:, :, 0, :])
                        nc.sync.dma_start(
                            out=p_s[rng].rearrange("(b r c) -> r b c", r=K, c=NT),
                            in_=tsb4[:, :, 1, :])

                # ---- phase 2: softplus + sigmoid over repacked [128, FLAT/128]
                W2 = FLAT // 128
                zt = ph2.tile([128, W2], f32, tag="zt")
                nc.sync.dma_start(out=zt[:], in_=z4_s[:].rearrange("(p x) -> p x", p=128))
                pt = ph2.tile([128, W2], f32, tag="pt")
                nc.sync.dma_start(out=pt[:], in_=p_s[:].rearrange("(p x) -> p x", p=128))
                e4 = ph2.tile([128, W2], f32, tag="e4")
                nc.scalar.activation(e4[:], zt[:], A.Exp)
                ysb = ph2.tile([128, W2], f32, tag="ysb")
                nc.scalar.activation(ysb[:], e4[:], A.Ln, bias=1.0)
                nc.sync.dma_start(out=y_out[:].rearrange("(p x) -> p x", p=128), in_=ysb[:])
                t4 = ph2.tile([128, W2], f32, tag="t4")
                nc.vector.tensor_sub(t4[:], zt[:], ysb[:])
                s4 = ph2.tile([128, W2], f32, tag="s4")
                nc.scalar.activation(s4[:], t4[:], A.Exp)
                yd = ph2.tile([128, W2], f32, tag="yd")
                nc.vector.scalar_tensor_tensor(yd[:], s4[:], -1.0, pt[:], Op.mult, Op.mult)
                nc.sync.dma_start(out=yd_out[:].rearrange("(p x) -> p x", p=128), in_=yd[:])

            if repeat == 1:
                body()
            else:
                with tc.For_i(0, repeat, 1):
                    body()

    nc.compile()
    return nc


def get_nc(repeat=1):
    if repeat not in _NC_CACHE:
        _NC_CACHE[repeat] = _build_nc(repeat)
    return _NC_CACHE[repeat]


def prep_inputs(hidden_states, time_delta_seqs, w1, b1, w2, b2, wA, bA, wB, bB,
                w3, b3):
    """Host-side constant folding + per-core sharding. Returns list of in_maps."""
    f32 = np.float32
    f64 = np.float64
    w2 = np.asarray(w2, f32)
    w2h = w2[:, :H]
    w2t = w2[:, H:]
    w1c = np.asarray(w1, f32)[:, 0].astype(f64)
    u1 = (w2t.astype(f64) @ w1c).astype(f32)
    c1 = (w2t.astype(f64) @ np.asarray(b1, f64) + np.asarray(b2, f64)).astype(f32)
    u1s = (u1.astype(f64) / BS).astype(f32)

    wA = np.asarray(wA, f32)
    wB = np.asarray(wB, f32)
    w3 = np.asarray(w3, f32)
    w2hT_a = np.ascontiguousarray(w2h.T).reshape(2, 128, H)
    wAT_a = np.ascontiguousarray(wA.T).reshape(2, 128, H)
    wBn2T_a = np.ascontiguousarray((-2.0 * wB.astype(f64)).astype(f32).T).reshape(2, 128, H)
    w3n2T_a = np.ascontiguousarray((-2.0 * w3.astype(f64)).astype(f32).T).reshape(2, 128, K)
    w3T_a = np.ascontiguousarray(w3.T).reshape(2, 128, K)
    u1c1_a = np.stack([c1, u1], axis=0).astype(f32)                      # [2, H]
    cB3_a = (wB.astype(f64).sum(axis=1) + np.asarray(bB, f64)).astype(f32).reshape(1, H)
    c34_a = (w3.astype(f64).sum(axis=1) + np.asarray(b3, f64)).astype(f32).reshape(1, K)
    nbA = (-2.0 * np.asarray(bA, f64)).astype(f32)
    ncB = (-2.0 * (wB.astype(f64).sum(axis=1) + np.asarray(bB, f64))).astype(f32)
    cvec_a = np.stack([nbA[:128], nbA[128:], ncB[:128], ncB[128:]], axis=1).astype(f32)
    u1s_a = np.stack([u1s[:128], u1s[128:]], axis=1).astype(f32)

    h_flat = np.asarray(hidden_states, f32).reshape(BS, H)
    td_flat = np.asarray(time_delta_seqs, f32).reshape(BS)

    in_maps = []
    for c in range(NCORES):
        sl = slice(c * TCORE, (c + 1) * TCORE)
        h_c = np.ascontiguousarray(h_flat[sl].T).reshape(2, 128, TCORE)
        tdo_c = np.stack([np.ones(TCORE, f32), td_flat[sl]], axis=0).astype(f32)
        in_maps.append(dict(
            h_t=h_c, tdo_in=tdo_c, w2hT=w2hT_a, wAT=wAT_a, wBn2T=wBn2T_a,
            w3n2T=w3n2T_a, w3T=w3T_a, u1c1=u1c1_a, cB3_d=cB3_a, c34_d=c34_a,
            cvec=cvec_a, u1s_d=u1s_a))
    return in_maps


def assemble_outputs(results):
    """results: list (per core) of dicts with y_out/yd_out flat arrays."""
    ys, yds = [], []
    for c in range(NCORES):
        y = np.asarray(results[c]["y_out"]).reshape(NBLK, K, NT)
        yd = np.asarray(results[c]["yd_out"]).reshape(NBLK, K, NT)
        ys.append(y.transpose(0, 2, 1).reshape(TCORE, K))
        yds.append(yd.transpose(0, 2, 1).reshape(TCORE, K))
    integral = np.concatenate(ys, axis=0).reshape(B, S, K).astype(np.float32)
    deriv = np.concatenate(yds, axis=0).reshape(B, S, K).astype(np.float32)
    return integral, deriv


def kernel(**inputs):
    from concourse.bass_utils import run_bass_kernel_spmd
    nc = get_nc(repeat=1)
    in_maps = prep_inputs(**inputs)
    res = run_bass_kernel_spmd(nc, in_maps, list(range(NCORES)))
    return assemble_outputs(res.results)



# revision 10
# speedup vs baseline: 1.3784x; 1.3784x over previous
"""Trainium2 Bass kernel for nn_CumulHazardFunctionNetwork.

Per token (b, s) with hidden state h [256] and time delta td:
    o1  = tanh(h @ w2h.T + td*u1 + c1)          u1 = w2t @ w1[:,0],  c1 = w2t@b1 + b2
    o2  = tanh(o1 @ wA.T + bA)
    o3  = tanh(o2 @ wB.T + bB)
    y   = softplus(o3 @ w3.T + b3)              (integral_lambda)
    yd  = sigmoid(z4) * (d3 @ w3.T) / (B*S)     (derivative_integral_lambda, JVP wrt td)
with the JVP chain d1 = u1s*(1-o1^2), d_{k+1} = tanh'(z_{k+1}) * (d_k @ W.T).

Mapping: pure data parallel over 8 NeuronCores (16384 tokens each),
feature-major layout ([feature, token]); weights replicated; fp32r matmuls.

z2, z3 >> 0 always (all-positive weights), so layers 2/3 use
    E = 4*exp(-2*z) = exp(-2*z + ln4),  tanh(z) = 1 - E/2,  tanh'(z) = E
(rel err < 3e-6), with the constants folded so every elementwise op is one
instruction: the exp bias absorbs ln4, the next-layer weights absorb the
-1/2 (wBzT = -wB.T/2, w3zT = -w3.T/2).  The u1s scale of d1 is folded into
the weights of the first JVP matmul (wApT = (wA*diag(u1s)).T), so instead of
d1 the kernel feeds s = o1^2 forward:
    WA@d1 = cA' - WA'@s      (cA' = rowsum(WA'))
    d2n  := -d2 = (G - cA') * E2,   G = WA'@s        (one DVE op)
    d3n  := -d3 = E3 * (WB @ d2n)                    (one DVE op)
softplus/sigmoid run in a second phase (one ACT table-set switch) on the
repacked [10, T] tail: y = ln(1+e^z4), sigma = exp(z4 - y).

Engine split per block: PE 26 matmuls; ACT tanh/E2/E3; Pool (gpsimd) the
o1^2 square; DVE the two PSUM-consuming mults + tail copy-outs.  Emission
is software-pipelined one block deep (z1 of block j+1 and tail of block j-1
issue inside round j) so the PE stream never waits on the ACT/DVE chain.
"""

import numpy as np

H, K, B, S = 256, 10, 32, 4096
BS = B * S
NCORES = 8
TCORE = BS // NCORES          # 16384 tokens per core
NT = 512                      # tokens per block (PSUM bank = 512 fp32)
NBLK = TCORE // NT            # 32
FLAT = NBLK * K * NT          # 163840 = 128 * 1280
TDC = 4                       # blocks per tdo chunk
HB = 2                        # blocks per h DMA
TB = 2                        # blocks per tail DMA

_NC_CACHE = {}


def _build_nc(repeat=1):
    import concourse.mybir as mybir
    import concourse.tile as tile
    from concourse import bacc

    dt = mybir.dt
    f32 = dt.float32
    f32r = dt.float32r
    A = mybir.ActivationFunctionType
    Op = mybir.AluOpType

    nc = bacc.Bacc("TRN2", target_bir_lowering=False, debug=False)

    h_t = nc.dram_tensor("h_t", [2, 128, TCORE], f32r, kind="ExternalInput")
    # row 0: ones, row 1: td  (rank-1 rhs for [c1; u1]; ones at partition 0)
    tdo_in = nc.dram_tensor("tdo_in", [2, TCORE], f32r, kind="ExternalInput")
    w2hT = nc.dram_tensor("w2hT", [2, 128, H], f32r, kind="ExternalInput")
    wAT = nc.dram_tensor("wAT", [2, 128, H], f32r, kind="ExternalInput")
    wApT = nc.dram_tensor("wApT", [2, 128, H], f32r, kind="ExternalInput")   # (wA*u1s).T
    wBzT = nc.dram_tensor("wBzT", [2, 128, H], f32r, kind="ExternalInput")   # (-wB/2).T
    wBT = nc.dram_tensor("wBT", [2, 128, H], f32r, kind="ExternalInput")
    w3zT = nc.dram_tensor("w3zT", [2, 128, K], f32r, kind="ExternalInput")   # (-w3/2).T
    w3T = nc.dram_tensor("w3T", [2, 128, K], f32r, kind="ExternalInput")
    u1c1 = nc.dram_tensor("u1c1", [2, H], f32r, kind="ExternalInput")        # [c1; u1]
    c34_d = nc.dram_tensor("c34_d", [K, 1], f32, kind="ExternalInput")       # rowsum(w3)+b3
    cvec = nc.dram_tensor("cvec", [128, 4], f32, kind="ExternalInput")       # exp biases per m
    cAp_d = nc.dram_tensor("cAp_d", [128, 2], f32, kind="ExternalInput")     # rowsum(wA*u1s) per m
    y_out = nc.dram_tensor("y_out", [FLAT], f32, kind="ExternalOutput")
    yd_out = nc.dram_tensor("yd_out", [FLAT], f32, kind="ExternalOutput")

    with tile.TileContext(nc) as tc:
        with tc.tile_pool(name="consts", bufs=1) as consts, \
             tc.tile_pool(name="hp", bufs=3) as hp, \
             tc.tile_pool(name="tdp", bufs=2) as tdp, \
             tc.tile_pool(name="ap", bufs=3) as ap, \
             tc.tile_pool(name="tailp", bufs=2) as tailp, \
             tc.tile_pool(name="ph2", bufs=1) as ph2, \
             tc.tile_pool(name="ps", bufs=4, space="PSUM") as psp, \
             tc.tile_pool(name="scr", bufs=1, space="DRAM") as scr:

            def body():
                def wtile(name, src, fdim):
                    t = consts.tile([128, 2, fdim], f32r, tag=name)
                    nc.sync.dma_start(out=t[:], in_=src[:].rearrange("k p f -> p k f"))
                    return t

                w2hT_sb = wtile("w2hT", w2hT, H)
                wAT_sb = wtile("wAT", wAT, H)
                wApT_sb = wtile("wApT", wApT, H)
                wBzT_sb = wtile("wBzT", wBzT, H)
                wBT_sb = wtile("wBT", wBT, H)
                w3zT_sb = wtile("w3zT", w3zT, K)
                w3T_sb = wtile("w3T", w3T, K)
                u1c1_sb = consts.tile([2, H], f32r, tag="u1c1")
                nc.sync.dma_start(out=u1c1_sb[:], in_=u1c1[:])
                c34_sb = consts.tile([K, 1], f32, tag="c34")
                nc.sync.dma_start(out=c34_sb[:], in_=c34_d[:])
                cvec_sb = consts.tile([128, 4], f32, tag="cvec")
                nc.sync.dma_start(out=cvec_sb[:], in_=cvec[:])
                cAp_sb = consts.tile([128, 2], f32, tag="cAp")
                nc.sync.dma_start(out=cAp_sb[:], in_=cAp_d[:])

                z4_s = scr.tile([FLAT], f32, tag="z4s")
                p_s = scr.tile([FLAT], f32, tag="ps_scr")

                state = {}

                def dma_h(c):
                    if c * HB >= NBLK:
                        return
                    t = hp.tile([128, 2, HB * NT], f32r, tag="h", name=f"h_{c}")
                    nc.sync.dma_start(
                        out=t[:],
                        in_=h_t[:, :, c * HB * NT:(c + 1) * HB * NT]
                            .rearrange("k p n -> p k n"))
                    state[("h", c)] = t

                def dma_td(c):
                    if c * TDC >= NBLK:
                        return
                    t = tdp.tile([2, TDC * NT], f32r, tag="tdo", name=f"tdo_{c}")
                    nc.sync.dma_start(
                        out=t[:], in_=tdo_in[:, c * TDC * NT:(c + 1) * TDC * NT])
                    state[("td", c)] = t

                def stage1(j):
                    """z1(j) matmuls + tanh + square; leaves o1/s in state."""
                    h_sb = state[("h", j // HB)]
                    tdo_sb = state[("td", j // TDC)]
                    hs = [h_sb[:, kk, (j % HB) * NT:(j % HB + 1) * NT]
                          for kk in range(2)]
                    tds = tdo_sb[:, (j % TDC) * NT:(j % TDC + 1) * NT]
                    z1 = psp.tile([128, 2, NT], f32, tag="ps")
                    for m in range(2):
                        ms = slice(m * 128, (m + 1) * 128)
                        nc.tensor.matmul(z1[:, m, :], w2hT_sb[:, 0, ms], hs[0],
                                         start=True, stop=False)
                        nc.tensor.matmul(z1[:, m, :], w2hT_sb[:, 1, ms], hs[1],
                                         start=False, stop=False)
                        nc.tensor.matmul(z1[:, m, :], u1c1_sb[:, ms], tds,
                                         start=False, stop=True)
                    o1 = ap.tile([128, 2, NT], f32r, tag="o1")
                    nc.scalar.activation(o1[:].rearrange("p k n -> p (k n)"),
                                         z1[:].rearrange("p k n -> p (k n)"), A.Tanh)
                    s = ap.tile([128, 2, NT], f32r, tag="s")
                    o1f = o1[:].rearrange("p k n -> p (k n)").bitcast(f32)
                    nc.gpsimd.tensor_tensor(
                        s[:].rearrange("p k n -> p (k n)"), o1f, o1f, Op.mult)
                    state[("o1", j)] = o1
                    state[("s", j)] = s

                def tail_pe(j):
                    """z4/p matmuls of block j (consume E3(j), d3n(j))."""
                    e3 = state.pop(("e3", j))
                    d3n = state.pop(("d3n", j))
                    tl = psp.tile([K, 2, NT], f32, tag="ps")
                    nc.tensor.matmul(tl[:, 0, :], w3zT_sb[:, 0, :], e3[:, 0, :],
                                     start=True, stop=False)
                    nc.tensor.matmul(tl[:, 0, :], w3zT_sb[:, 1, :], e3[:, 1, :],
                                     start=False, stop=True)
                    nc.tensor.matmul(tl[:, 1, :], w3T_sb[:, 0, :], d3n[:, 0, :],
                                     start=True, stop=False)
                    nc.tensor.matmul(tl[:, 1, :], w3T_sb[:, 1, :], d3n[:, 1, :],
                                     start=False, stop=True)
                    state[("tl", j)] = tl

                def tail_copy(j):
                    """copy-out of block j's tail (z4 gets +c34), DMA per TB."""
                    tl = state.pop(("tl", j))
                    if j % TB == 0:
                        state["tsb4"] = tailp.tile([K, TB, 2, NT], f32, tag="tail", name=f"tail_{j}")
                    tsb4 = state["tsb4"]
                    nc.vector.tensor_scalar(tsb4[:, j % TB, 0, :], tl[:, 0, :],
                                            c34_sb[:, 0:1], None, Op.add)
                    nc.vector.tensor_copy(tsb4[:, j % TB, 1, :], tl[:, 1, :])
                    if j % TB == TB - 1:
                        j0 = j - (TB - 1)
                        rng = slice(j0 * K * NT, (j0 + TB) * K * NT)
                        nc.sync.dma_start(
                            out=z4_s[rng].rearrange("(b r c) -> r b c", r=K, c=NT),
                            in_=tsb4[:, :, 0, :])
                        nc.sync.dma_start(
                            out=p_s[rng].rearrange("(b r c) -> r b c", r=K, c=NT),
                            in_=tsb4[:, :, 1, :])

                def mid(j):
                    """layers 2+3 of block j (consume o1(j), s(j))."""
                    o1 = state.pop(("o1", j))
                    s = state.pop(("s", j))
                    # ---- layer 2: z2 = o1 @ wA.T ; G = s @ wAp.T
                    z2 = psp.tile([128, 2, NT], f32, tag="ps")
                    g2 = psp.tile([128, 2, NT], f32, tag="ps")
                    for m in range(2):
                        ms = slice(m * 128, (m + 1) * 128)
                        nc.tensor.matmul(z2[:, m, :], wAT_sb[:, 0, ms], o1[:, 0, :],
                                         start=True, stop=False)
                        nc.tensor.matmul(z2[:, m, :], wAT_sb[:, 1, ms], o1[:, 1, :],
                                         start=False, stop=True)
                        nc.tensor.matmul(g2[:, m, :], wApT_sb[:, 0, ms], s[:, 0, :],
                                         start=True, stop=False)
                        nc.tensor.matmul(g2[:, m, :], wApT_sb[:, 1, ms], s[:, 1, :],
                                         start=False, stop=True)
                    e2 = ap.tile([128, 2, NT], f32r, tag="e2")
                    for m in range(2):
                        nc.scalar.activation(e2[:, m, :], z2[:, m, :], A.Exp,
                                             scale=-2.0, bias=cvec_sb[:, m:m + 1])
                    # d2n = (G - cA') * E2
                    d2n = ap.tile([128, 2, NT], f32r, tag="d2n")
                    for m in range(2):
                        nc.vector.scalar_tensor_tensor(
                            d2n[:, m, :], g2[:, m, :], cAp_sb[:, m:m + 1],
                            e2[:, m, :].bitcast(f32), Op.subtract, Op.mult)

                    # ---- layer 3: z3 = -E2 @ wB.T/2 ; g3n = d2n @ wB.T
                    z3 = psp.tile([128, 2, NT], f32, tag="ps")
                    g3 = psp.tile([128, 2, NT], f32, tag="ps")
                    for m in range(2):
                        ms = slice(m * 128, (m + 1) * 128)
                        nc.tensor.matmul(z3[:, m, :], wBzT_sb[:, 0, ms], e2[:, 0, :],
                                         start=True, stop=False)
                        nc.tensor.matmul(z3[:, m, :], wBzT_sb[:, 1, ms], e2[:, 1, :],
                                         start=False, stop=True)
                        nc.tensor.matmul(g3[:, m, :], wBT_sb[:, 0, ms], d2n[:, 0, :],
                                         start=True, stop=False)
                        nc.tensor.matmul(g3[:, m, :], wBT_sb[:, 1, ms], d2n[:, 1, :],
                                         start=False, stop=True)
                    e3 = ap.tile([128, 2, NT], f32r, tag="e3")
                    for m in range(2):
                        nc.scalar.activation(e3[:, m, :], z3[:, m, :], A.Exp,
                                             scale=-2.0, bias=cvec_sb[:, 2 + m:3 + m])
                    # d3n = E3 * g3n
                    d3n = ap.tile([128, 2, NT], f32r, tag="d3n")
                    for m in range(2):
                        nc.vector.tensor_tensor(
                            d3n[:, m, :], e3[:, m, :].bitcast(f32), g3[:, m, :],
                            Op.mult)
                    state[("e3", j)] = e3
                    state[("d3n", j)] = d3n

                # ---- software-pipelined main loop (1 block stagger each way)
                dma_h(0)
                dma_h(1)
                dma_h(2)
                dma_td(0)
                dma_td(1)
                stage1(0)
                for j in range(NBLK):
                    # prefetch: chunk c's buffer is reused only after the
                    # last z1 read of chunk c-bufs (emitted in an earlier round)
                    if j % HB == 1:
                        dma_h((j + 5) // HB)
                    if j % TDC == 3:
                        dma_td((j + 5) // TDC)
                    if j + 1 < NBLK:
                        stage1(j + 1)
                    if j - 1 >= 0:
                        tail_pe(j - 1)
                        tail_copy(j - 1)
                    mid(j)
                tail_pe(NBLK - 1)
                tail_copy(NBLK - 1)

                # ---- phase 2: softplus + sigmoid over repacked [128, FLAT/128]
                W2 = FLAT // 128
                zt = ph2.tile([128, W2], f32, tag="zt")
                nc.sync.dma_start(out=zt[:], in_=z4_s[:].rearrange("(p x) -> p x", p=128))
                pt = ph2.tile([128, W2], f32, tag="pt")
                nc.sync.dma_start(out=pt[:], in_=p_s[:].rearrange("(p x) -> p x", p=128))
                e4 = ph2.tile([128, W2], f32, tag="e4")
                nc.scalar.activation(e4[:], zt[:], A.Exp)
                ysb = ph2.tile([128, W2], f32, tag="ysb")
                nc.scalar.activation(ysb[:], e4[:], A.Ln, bias=1.0)
                nc.sync.dma_start(out=y_out[:].rearrange("(p x) -> p x", p=128), in_=ysb[:])
                t4 = ph2.tile([128, W2], f32, tag="t4")
                nc.vector.tensor_sub(t4[:], zt[:], ysb[:])
                s4 = ph2.tile([128, W2], f32, tag="s4")
                nc.scalar.activation(s4[:], t4[:], A.Exp)
                yd = ph2.tile([128, W2], f32, tag="yd")
                nc.vector.scalar_tensor_tensor(yd[:], s4[:], -1.0, pt[:], Op.mult, Op.mult)
                nc.sync.dma_start(out=yd_out[:].rearrange("(p x) -> p x", p=128), in_=yd[:])

            if repeat == 1:
                body()
            else:
                with tc.For_i(0, repeat, 1):
                    body()

    nc.compile()
    return nc


def get_nc(repeat=1):
    if repeat not in _NC_CACHE:
        _NC_CACHE[repeat] = _build_nc(repeat)
    return _NC_CACHE[repeat]


def prep_inputs(hidden_states, time_delta_seqs, w1, b1, w2, b2, wA, bA, wB, bB,
                w3, b3):
    """Host-side constant folding + per-core sharding. Returns list of in_maps."""
    f32 = np.float32
    f64 = np.float64
    ln4 = float(np.log(4.0))
    w2 = np.asarray(w2, f32)
    w2h = w2[:, :H]
    w2t = w2[:, H:]
    w1c = np.asarray(w1, f32)[:, 0].astype(f64)
    u1 = (w2t.astype(f64) @ w1c).astype(f32)
    c1 = (w2t.astype(f64) @ np.asarray(b1, f64) + np.asarray(b2, f64)).astype(f32)
    u1s = (u1.astype(f64) / BS).astype(f32)

    wA = np.asarray(wA, f32)
    wB = np.asarray(wB, f32)
    w3 = np.asarray(w3, f32)
    wAp = (wA.astype(f64) * u1s.astype(f64)[None, :]).astype(f32)
    cAp = wAp.astype(f64).sum(axis=1).astype(f32)
    w2hT_a = np.ascontiguousarray(w2h.T).reshape(2, 128, H)
    wAT_a = np.ascontiguousarray(wA.T).reshape(2, 128, H)
    wApT_a = np.ascontiguousarray(wAp.T).reshape(2, 128, H)
    wBzT_a = np.ascontiguousarray((-0.5 * wB.astype(f64)).astype(f32).T).reshape(2, 128, H)
    wBT_a = np.ascontiguousarray(wB.T).reshape(2, 128, H)
    w3zT_a = np.ascontiguousarray((-0.5 * w3.astype(f64)).astype(f32).T).reshape(2, 128, K)
    w3T_a = np.ascontiguousarray(w3.T).reshape(2, 128, K)
    u1c1_a = np.stack([c1, u1], axis=0).astype(f32)                      # [2, H]
    c34_a = (w3.astype(f64).sum(axis=1) + np.asarray(b3, f64)).astype(f32).reshape(K, 1)
    nbA = (-2.0 * np.asarray(bA, f64) + ln4).astype(f32)
    ncB = (-2.0 * (wB.astype(f64).sum(axis=1) + np.asarray(bB, f64)) + ln4).astype(f32)
    cvec_a = np.stack([nbA[:128], nbA[128:], ncB[:128], ncB[128:]], axis=1).astype(f32)
    cAp_a = np.stack([cAp[:128], cAp[128:]], axis=1).astype(f32)

    h_flat = np.asarray(hidden_states, f32).reshape(BS, H)
    td_flat = np.asarray(time_delta_seqs, f32).reshape(BS)

    in_maps = []
    for c in range(NCORES):
        sl = slice(c * TCORE, (c + 1) * TCORE)
        h_c = np.ascontiguousarray(h_flat[sl].T).reshape(2, 128, TCORE)
        tdo_c = np.stack([np.ones(TCORE, f32), td_flat[sl]], axis=0).astype(f32)
        in_maps.append(dict(
            h_t=h_c, tdo_in=tdo_c, w2hT=w2hT_a, wAT=wAT_a, wApT=wApT_a,
            wBzT=wBzT_a, wBT=wBT_a, w3zT=w3zT_a, w3T=w3T_a, u1c1=u1c1_a,
            c34_d=c34_a, cvec=cvec_a, cAp_d=cAp_a))
    return in_maps


def assemble_outputs(results):
    """results: list (per core) of dicts with y_out/yd_out flat arrays."""
    ys, yds = [], []
    for c in range(NCORES):
        y = np.asarray(results[c]["y_out"]).reshape(NBLK, K, NT)
        yd = np.asarray(results[c]["yd_out"]).reshape(NBLK, K, NT)
        ys.append(y.transpose(0, 2, 1).reshape(TCORE, K))
        yds.append(yd.transpose(0, 2, 1).reshape(TCORE, K))
    integral = np.concatenate(ys, axis=0).reshape(B, S, K).astype(np.float32)
    deriv = np.concatenate(yds, axis=0).reshape(B, S, K).astype(np.float32)
    return integral, deriv


def kernel(**inputs):
    from concourse.bass_utils import run_bass_kernel_spmd
    nc = get_nc(repeat=1)
    in_maps = prep_inputs(**inputs)
    res = run_bass_kernel_spmd(nc, in_maps, list(range(NCORES)))
    return assemble_outputs(res.results)
